# revision 31
# baseline (speedup 1.0000x reference)
"""Trainium2 Bass kernel for nn_MultiHeadAttention_78847009620030.

Banded (|i-j| <= 128) multi-head attention, B=4, K=2048, D=512, H=8, dh=64.

Sharding: 8 cores = (batch b in 0..3) x (sequence half in 0..1). Each core
computes 1024 query rows of one batch; key/value rows come with a 128-row
halo on each side (zero-padded at sequence ends), so no collectives are
needed. All sharding/unsharding happens on host inside kernel().

Per-core kernel (one NEFF, SPMD on cores 0-7), all matmuls in bf16:
  - Host pre-transposes activations/weights so no on-chip input transposes
    are needed; host also casts matmul operands to bf16 (halves DMA bytes).
  - Q/K projections produce transposed activations [feature, seq]; V
    projection produces [seq, (head, 65)] where column 64 per head is a
    row-validity flag (kills the zero-padded halo rows in the softmax
    denominator and keeps one NEFF valid for all cores).
  - Scores for a 128-query tile are 3 matmuls into one [128, 384] PSUM bank
    ([k-within-block, (window, q)] layout); one Exp activation (scale folded
    in) and one multiplicative band-mask (constant [128,384] 0/1 tile) per
    (head, q-tile). Softmax skips max-subtraction (scores are tiny; exact
    same math as the reference up to fp rounding).
  - P~ @ V_aug yields unnormalized attention AND the softmax denominator in
    one PSUM accumulation; normalization is one per-partition divide fused
    into the PSUM->SBUF move.
  - Even/odd head pairs issue score matmuls back-to-back with contraction
    rows 0-63 / 64-127, so the PE runs them concurrently (row tiling).
  - The output projection consumes PE-transposed attention tiles; final
    bias-add is fused into the PSUM->SBUF move.
"""

import threading

import numpy as np
import ml_dtypes

import concourse.bass as bass
import concourse.bacc as bacc
import concourse.mybir as mybir
import concourse.tile as tile
from concourse.bass_utils import run_bass_kernel_spmd

# Problem shape (hardcoded per contest contract).
B, K, D = 4, 2048, 512
H, DH = 8, 64
ATTN = 128
HALF = 1024            # query rows per core
KV = HALF + 2 * ATTN   # 1280 key/value rows per core (with halo)
NCORES = 8
NFT = D // 128         # 4 feature tiles
NT_KV = KV // 128      # 10 kv row blocks
NTQ = HALF // 128      # 8 query tiles of 128
SCALE = float(1.0 / np.sqrt(np.float32(K)))
HV = DH + 1            # 65: head value dim + validity column

F32 = mybir.dt.float32
BF16 = mybir.dt.bfloat16
NP_BF16 = ml_dtypes.bfloat16


def _bcast(ap, p=128):
    """Partition-broadcast a 1-D DRAM AP to [p, n] for DMA."""
    return bass.AP(tensor=ap.tensor, offset=ap.offset, ap=[[0, p], list(ap.ap[0])])


def build_nc():
    nc = bacc.Bacc()

    xq = nc.dram_tensor("xqT", [D, HALF], BF16, kind="ExternalInput")
    xk = nc.dram_tensor("xkT", [D, KV], BF16, kind="ExternalInput")
    xv = nc.dram_tensor("xvT", [D, KV], BF16, kind="ExternalInput")
    wq = nc.dram_tensor("wqT", [D, D], BF16, kind="ExternalInput")
    wk = nc.dram_tensor("wkT", [D, D], BF16, kind="ExternalInput")
    wv = nc.dram_tensor("wvT", [D, H * HV], BF16, kind="ExternalInput")
    wo = nc.dram_tensor("woT", [D, D], BF16, kind="ExternalInput")
    bq = nc.dram_tensor("bq", [D], F32, kind="ExternalInput")
    bk = nc.dram_tensor("bk", [D], F32, kind="ExternalInput")
    bv = nc.dram_tensor("bv", [H * HV], F32, kind="ExternalInput")
    bo = nc.dram_tensor("bo", [D], F32, kind="ExternalInput")
    valid = nc.dram_tensor("valid", [KV], F32, kind="ExternalInput")
    out = nc.dram_tensor("out", [HALF, D], F32, kind="ExternalOutput")

    with tile.TileContext(nc) as tc:
        with (
            tc.tile_pool(name="const", bufs=1) as const,
            tc.tile_pool(name="xin", bufs=1) as xin,
            tc.tile_pool(name="acts", bufs=1) as acts,
            tc.tile_pool(name="work", bufs=1) as work,
            tc.tile_pool(name="ps", bufs=1, space="PSUM") as ps,
        ):
            # ---- weights + inputs, in consumption order for fast PE start ----
            wq_sb = const.tile([128, NFT, D], BF16, tag="wq")
            rr = wq[:].rearrange("(dt p) f -> p dt f", p=128)
            for dt_i in range(NFT):
                nc.sync.dma_start(wq_sb[:, dt_i], rr[:, dt_i])
            xq_sb = xin.tile([128, NFT, HALF], BF16, tag="xq")
            rr = xq[:].rearrange("(dt p) s -> p dt s", p=128)
            for dt_i in range(NFT):
                nc.scalar.dma_start(xq_sb[:, dt_i], rr[:, dt_i])

            bq_sb = const.tile([128, NFT], F32, tag="bqs")
            bk_sb = const.tile([128, NFT], F32, tag="bks")
            nc.sync.dma_start(bq_sb, bq[:].rearrange("(ft p) -> p ft", p=128))
            nc.sync.dma_start(bk_sb, bk[:].rearrange("(ft p) -> p ft", p=128))

            wk_sb = const.tile([128, NFT, D], BF16, tag="wk")
            rr = wk[:].rearrange("(dt p) f -> p dt f", p=128)
            for dt_i in range(NFT):
                nc.sync.dma_start(wk_sb[:, dt_i], rr[:, dt_i])
            xk_sb = xin.tile([128, NFT, KV], BF16, tag="xk")
            rr = xk[:].rearrange("(dt p) s -> p dt s", p=128)
            for dt_i in range(NFT):
                nc.sync.dma_start(xk_sb[:, dt_i], rr[:, dt_i])

            wv_sb = const.tile([128, NFT, H * HV], BF16, tag="wv")
            rr = wv[:].rearrange("(dt p) f -> p dt f", p=128)
            for dt_i in range(NFT):
                nc.sync.dma_start(wv_sb[:, dt_i], rr[:, dt_i])
            xv_sb = xin.tile([128, NFT, KV], BF16, tag="xv")
            rr = xv[:].rearrange("(dt p) s -> p dt s", p=128)
            for dt_i in range(NFT):
                nc.sync.dma_start(xv_sb[:, dt_i], rr[:, dt_i])

            wo_sb = const.tile([128, NFT, D], BF16, tag="wo")
            nc.sync.dma_start(wo_sb, wo[:].rearrange("(dt p) f -> p dt f", p=128))

            bv_bc = const.tile([128, H * HV], F32, tag="bvb")
            bo_bc = const.tile([128, D], F32, tag="bob")
            nc.sync.dma_start(bv_bc, _bcast(bv[:]))
            nc.sync.dma_start(bo_bc, _bcast(bo[:]))
            valid_sb = const.tile([128, NT_KV], F32, tag="vld")
            nc.sync.dma_start(valid_sb, valid[:].rearrange("(b p) -> p b", p=128))

            # identity for PE transpose (bf16)
            ident = const.tile([128, 128], BF16, tag="ident")
            nc.gpsimd.memset(ident, 0.0)
            nc.gpsimd.affine_select(
                out=ident, in_=ident,
                compare_op=mybir.AluOpType.not_equal, fill=1.0,
                base=0, pattern=[[-1, 128]], channel_multiplier=1,
            )

            # constant band masks for window blocks 0 and 2 (block 1 is
            # fully in-band): [k-in-block(128), (w in {0,2}, q)(2x128)] bf16
            bmask = const.tile([128, 2, 128], BF16, tag="bmask")
            nc.gpsimd.memset(bmask, 1.0)
            nc.gpsimd.affine_select(
                out=bmask[:, 0], in_=bmask[:, 0],
                compare_op=mybir.AluOpType.is_ge, fill=0.0,
                base=0, pattern=[[-1, 128]], channel_multiplier=1,
            )
            nc.gpsimd.affine_select(
                out=bmask[:, 1], in_=bmask[:, 1],
                compare_op=mybir.AluOpType.is_ge, fill=0.0,
                base=0, pattern=[[1, 128]], channel_multiplier=-1,
            )

            # ---- projections ----
            QT = acts.tile([128, NFT, HALF], BF16, tag="QT")
            KT = acts.tile([128, NFT, KV], BF16, tag="KT")
            VST = acts.tile([128, NT_KV, H * HV], BF16, tag="VST")

            for x_sb, w_sb, o_sb, b_sb, n_s in (
                (xq_sb, wq_sb, QT, bq_sb, HALF),
                (xk_sb, wk_sb, KT, bk_sb, KV),
            ):
                for ft in range(NFT):
                    for c0 in range(0, n_s, 512):
                        cw = min(512, n_s - c0)
                        acc = ps.tile([128, 512], F32, tag="ps_big", bufs=2)
                        for dt_i in range(NFT):
                            nc.tensor.matmul(
                                acc[:, :cw],
                                w_sb[:, dt_i, ft * 128:(ft + 1) * 128],
                                x_sb[:, dt_i, c0:c0 + cw],
                                start=(dt_i == 0),
                                stop=(dt_i == NFT - 1),
                            )
                        nc.vector.tensor_scalar_add(
                            o_sb[:, ft, c0:c0 + cw], acc[:, :cw], b_sb[:, ft:ft + 1]
                        )

            for blk in range(NT_KV):
                for hc in range(2):
                    c0 = hc * (H * HV // 2)  # 260-wide halves
                    cw = H * HV // 2
                    acc = ps.tile([128, 260], F32, tag="ps_big", bufs=2, name="acc")
                    for dt_i in range(NFT):
                        nc.tensor.matmul(
                            acc,
                            xv_sb[:, dt_i, blk * 128:(blk + 1) * 128],
                            wv_sb[:, dt_i, c0:c0 + cw],
                            start=(dt_i == 0),
                            stop=(dt_i == NFT - 1),
                        )
                    nc.vector.tensor_add(
                        VST[:, blk, c0:c0 + cw], acc, bv_bc[:, c0:c0 + cw]
                    )
                # validity column (per head): col h*HV + 64
                vcol = VST[:, blk].rearrange("p (h c) -> p h c", c=HV)[:, :, DH:DH + 1]
                nc.vector.tensor_scalar_add(vcol, vcol, valid_sb[:, blk:blk + 1])

            # ---- attention: q-tiles of 128, window = kv blocks t, t+1, t+2 ----
            for t in range(NTQ):
                attn_sb = work.tile([128, D], BF16, tag="attn", bufs=4, name="attn")
                for hp in range(H // 2):  # head pairs (2hp, 2hp+1) share ft plane
                    ft = hp
                    sts = []
                    for hs in range(2):
                        sts.append(ps.tile([128, 384], F32, tag="ps_st", bufs=4,
                                           name="st"))
                    # paired score MMs: rows 0-63 and 64-127
                    for w in range(3):
                        for hs in range(2):
                            p0 = hs * 64
                            nc.tensor.matmul(
                                sts[hs][:, w * 128:(w + 1) * 128],
                                KT[p0:p0 + 64, ft, (t + w) * 128:(t + w + 1) * 128],
                                QT[p0:p0 + 64, ft, t * 128:(t + 1) * 128],
                                start=True,
                                stop=True,
                                tile_position=(p0, 0),
                            )
                    aps2 = ps.tile([128, 2 * HV], F32, tag="ps_sm", bufs=2,
                                   name="aps2")
                    for hs in range(2):
                        h = 2 * hp + hs
                        pt = work.tile([128, 384], BF16, tag="pt", bufs=8)
                        nc.scalar.activation(
                            pt, sts[hs], mybir.ActivationFunctionType.Exp,
                            scale=SCALE,
                        )
                        ptv = pt.rearrange("p (w j) -> p w j", j=128)[:, 0:3:2, :]
                        if hs == 0:
                            nc.vector.tensor_mul(ptv, ptv, bmask)
                        else:
                            nc.gpsimd.tensor_mul(ptv, ptv, bmask)
                        for w in range(3):
                            nc.tensor.matmul(
                                aps2[:, hs * HV:hs * HV + HV],
                                pt[:, w * 128:(w + 1) * 128],
                                VST[:, t + w, h * HV:(h + 1) * HV],
                                start=(w == 0),
                                stop=(w == 2),
                            )
                    # normalize both heads: one batched reciprocal, two muls
                    rec2 = work.tile([128, 2], F32, tag="rec", bufs=8)
                    dn = aps2.rearrange("p (h c) -> p h c", c=HV)[:, :, DH:DH + 1]
                    nc.vector.reciprocal(rec2, dn)
                    for hs in range(2):
                        h = 2 * hp + hs
                        nc.vector.tensor_scalar_mul(
                            attn_sb[:, h * DH:(h + 1) * DH],
                            aps2[:, hs * HV:hs * HV + DH], rec2[:, hs:hs + 1]
                        )
                # transpose attn + output projection
                atT = []
                for ft in range(NFT):
                    tp = ps.tile([128, 128], BF16, tag="ps_sm", bufs=2, name="tp")
                    nc.tensor.transpose(tp, attn_sb[:, ft * 128:(ft + 1) * 128], ident)
                    at = work.tile([128, 128], BF16, tag="atT", bufs=8, name="at")
                    nc.vector.tensor_copy(at, tp)
                    atT.append(at)
                ops = ps.tile([128, D], F32, tag="ps_big", bufs=2, name="ops")
                for ft in range(NFT):
                    nc.tensor.matmul(
                        ops, atT[ft], wo_sb[:, ft, :],
                        start=(ft == 0), stop=(ft == NFT - 1),
                    )
                ob = work.tile([128, D], F32, tag="ob", bufs=3)
                nc.vector.tensor_add(ob, ops, bo_bc)
                nc.sync.dma_start(out[t * 128:(t + 1) * 128, :], ob)

    nc.compile()
    return nc


_CACHE = threading.Lock(), {}


def _get_nc():
    lock, cache = _CACHE
    with lock:
        if "nc" not in cache:
            cache["nc"] = build_nc()
        return cache["nc"]


def _prep_core_inputs(b, h2, query, key, value, shared):
    qs = h2 * HALF
    xqT = np.ascontiguousarray(query[b, qs:qs + HALF, :].T.astype(NP_BF16))

    lo, hi = qs - ATTN, qs + HALF + ATTN
    clo, chi = max(lo, 0), min(hi, K)
    pad_front, pad_back = clo - lo, hi - chi
    kpad = np.zeros((KV, D), NP_BF16)
    vpad = np.zeros((KV, D), NP_BF16)
    kpad[pad_front:KV - pad_back] = key[b, clo:chi, :].astype(NP_BF16)
    vpad[pad_front:KV - pad_back] = value[b, clo:chi, :].astype(NP_BF16)
    val = np.zeros(KV, np.float32)
    val[pad_front:KV - pad_back] = 1.0

    m = {
        "xqT": xqT,
        "xkT": np.ascontiguousarray(kpad.T),
        "xvT": np.ascontiguousarray(vpad.T),
        "valid": val,
    }
    m.update(shared)
    return m


def _prep_shared(Wq, bq, Wk, bk, Wv, bv, Wo, bo):
    wvT = np.asarray(Wv, np.float32).T  # [D, H*DH]
    wvT_aug = np.zeros((D, H * HV), np.float32)
    bv_aug = np.zeros(H * HV, np.float32)
    for h in range(H):
        wvT_aug[:, h * HV:h * HV + DH] = wvT[:, h * DH:(h + 1) * DH]
        bv_aug[h * HV:h * HV + DH] = np.asarray(bv, np.float32)[h * DH:(h + 1) * DH]
    return {
        "wqT": np.ascontiguousarray(np.asarray(Wq, np.float32).T.astype(NP_BF16)),
        "wkT": np.ascontiguousarray(np.asarray(Wk, np.float32).T.astype(NP_BF16)),
        "wvT": wvT_aug.astype(NP_BF16),
        "woT": np.ascontiguousarray(np.asarray(Wo, np.float32).T.astype(NP_BF16)),
        "bq": np.asarray(bq, np.float32),
        "bk": np.asarray(bk, np.float32),
        "bv": bv_aug,
        "bo": np.asarray(bo, np.float32),
    }


def kernel(query, key, value, Wq, bq, Wk, bk, Wv, bv, Wo, bo):
    query = np.asarray(query, np.float32)
    key = np.asarray(key, np.float32)
    value = np.asarray(value, np.float32)
    shared = _prep_shared(Wq, bq, Wk, bk, Wv, bv, Wo, bo)

    in_maps = []
    for core in range(NCORES):
        b, h2 = divmod(core, 2)
        in_maps.append(_prep_core_inputs(b, h2, query, key, value, shared))

    nc = _get_nc()
    res = run_bass_kernel_spmd(nc, in_maps, core_ids=list(range(NCORES)))

    full = np.empty((B, K, D), np.float32)
    for core in range(NCORES):
        b, h2 = divmod(core, 2)
        full[b, h2 * HALF:(h2 + 1) * HALF, :] = res.results[core]["out"]
    return full


# revision 32
# speedup vs baseline: 1.1290x; 1.1290x over previous
"""Trainium2 Bass kernel for nn_MultiHeadAttention_78847009620030.

Banded (|i-j| <= 128) multi-head attention, B=4, K=2048, D=512, H=8, dh=64.

Sharding: 8 cores = (batch b in 0..3) x (sequence half in 0..1). Each core
computes 1024 query rows of one batch; key/value rows come with a 128-row
halo on each side (zero-padded at sequence ends), so no collectives are
needed. All sharding/unsharding happens on host inside kernel().

Per-core kernel (one NEFF, SPMD on cores 0-7), all matmuls in bf16:
  - Host pre-transposes activations/weights so no on-chip input transposes
    are needed; host also casts matmul operands to bf16 (halves DMA bytes).
  - Q/K projections produce transposed activations [feature, seq]; V
    projection produces [seq, (head, 65)] where column 64 per head is a
    row-validity flag (kills the zero-padded halo rows in the softmax
    denominator and keeps one NEFF valid for all cores).
  - Scores for a 128-query tile are 3 matmuls into one [128, 384] PSUM bank
    ([k-within-block, (window, q)] layout); one Exp activation (scale folded
    in) and one multiplicative band-mask (constant [128,384] 0/1 tile) per
    (head, q-tile). Softmax skips max-subtraction (scores are tiny; exact
    same math as the reference up to fp rounding).
  - P~ @ V_aug yields unnormalized attention AND the softmax denominator in
    one PSUM accumulation; normalization is one per-partition divide fused
    into the PSUM->SBUF move.
  - Even/odd head pairs issue score matmuls back-to-back with contraction
    rows 0-63 / 64-127, so the PE runs them concurrently (row tiling).
  - The output projection consumes PE-transposed attention tiles; final
    bias-add is fused into the PSUM->SBUF move.
"""

import threading

import numpy as np
import ml_dtypes

import concourse.bass as bass
import concourse.bacc as bacc
import concourse.mybir as mybir
import concourse.tile as tile
from concourse.bass_utils import run_bass_kernel_spmd

# Problem shape (hardcoded per contest contract).
B, K, D = 4, 2048, 512
H, DH = 8, 64
ATTN = 128
HALF = 1024            # query rows per core
KV = HALF + 2 * ATTN   # 1280 key/value rows per core (with halo)
NCORES = 8
NFT = D // 128         # 4 feature tiles
NT_KV = KV // 128      # 10 kv row blocks
NTQ = HALF // 128      # 8 query tiles of 128
SCALE = float(1.0 / np.sqrt(np.float32(K)))
HV = DH + 1            # 65: head value dim + validity column

F32 = mybir.dt.float32
BF16 = mybir.dt.bfloat16
NP_BF16 = ml_dtypes.bfloat16


def _bcast(ap, p=128):
    """Partition-broadcast a 1-D DRAM AP to [p, n] for DMA."""
    return bass.AP(tensor=ap.tensor, offset=ap.offset, ap=[[0, p], list(ap.ap[0])])


def build_nc():
    nc = bacc.Bacc()

    xq = nc.dram_tensor("xqT", [D, HALF], BF16, kind="ExternalInput")
    xk = nc.dram_tensor("xkT", [D, KV], BF16, kind="ExternalInput")
    xv = nc.dram_tensor("xvT", [D, KV], BF16, kind="ExternalInput")
    wq = nc.dram_tensor("wqT", [D, D], BF16, kind="ExternalInput")
    wk = nc.dram_tensor("wkT", [D, D], BF16, kind="ExternalInput")
    wv = nc.dram_tensor("wvT", [D, H * HV], BF16, kind="ExternalInput")
    wo = nc.dram_tensor("woT", [D, D], BF16, kind="ExternalInput")
    bq = nc.dram_tensor("bq", [D], F32, kind="ExternalInput")
    bk = nc.dram_tensor("bk", [D], F32, kind="ExternalInput")
    bv = nc.dram_tensor("bv", [H * HV], F32, kind="ExternalInput")
    bo = nc.dram_tensor("bo", [D], F32, kind="ExternalInput")
    valid = nc.dram_tensor("valid", [KV], F32, kind="ExternalInput")
    out = nc.dram_tensor("out", [HALF, D], F32, kind="ExternalOutput")

    with tile.TileContext(nc) as tc:
        with (
            tc.tile_pool(name="const", bufs=1) as const,
            tc.tile_pool(name="xin", bufs=1) as xin,
            tc.tile_pool(name="acts", bufs=1) as acts,
            tc.tile_pool(name="work", bufs=1) as work,
            tc.tile_pool(name="ps", bufs=1, space="PSUM") as ps,
        ):
            # ---- weights + inputs, in consumption order for fast PE start ----
            wq_sb = const.tile([128, NFT, D], BF16, tag="wq")
            rr = wq[:].rearrange("(dt p) f -> p dt f", p=128)
            for dt_i in range(NFT):
                nc.sync.dma_start(wq_sb[:, dt_i], rr[:, dt_i])
            xq_sb = xin.tile([128, NFT, HALF], BF16, tag="xq")
            rr = xq[:].rearrange("(dt p) s -> p dt s", p=128)
            for dt_i in range(NFT):
                nc.scalar.dma_start(xq_sb[:, dt_i], rr[:, dt_i])

            bq_sb = const.tile([128, NFT], F32, tag="bqs")
            bk_sb = const.tile([128, NFT], F32, tag="bks")
            nc.sync.dma_start(bq_sb, bq[:].rearrange("(ft p) -> p ft", p=128))
            nc.sync.dma_start(bk_sb, bk[:].rearrange("(ft p) -> p ft", p=128))

            wk_sb = const.tile([128, NFT, D], BF16, tag="wk")
            rr = wk[:].rearrange("(dt p) f -> p dt f", p=128)
            for dt_i in range(NFT):
                nc.sync.dma_start(wk_sb[:, dt_i], rr[:, dt_i])
            xk_sb = xin.tile([128, NFT, KV], BF16, tag="xk")
            rr = xk[:].rearrange("(dt p) s -> p dt s", p=128)
            for dt_i in range(NFT):
                nc.sync.dma_start(xk_sb[:, dt_i], rr[:, dt_i])

            wv_sb = const.tile([128, NFT, H * HV], BF16, tag="wv")
            rr = wv[:].rearrange("(dt p) f -> p dt f", p=128)
            for dt_i in range(NFT):
                nc.sync.dma_start(wv_sb[:, dt_i], rr[:, dt_i])
            xv_sb = xin.tile([128, NFT, KV], BF16, tag="xv")
            rr = xv[:].rearrange("(dt p) s -> p dt s", p=128)
            for dt_i in range(NFT):
                nc.sync.dma_start(xv_sb[:, dt_i], rr[:, dt_i])

            wo_sb = const.tile([128, NFT, D], BF16, tag="wo")
            nc.sync.dma_start(wo_sb, wo[:].rearrange("(dt p) f -> p dt f", p=128))

            bv_bc = const.tile([128, H * HV], F32, tag="bvb")
            bo_bc = const.tile([128, D], F32, tag="bob")
            nc.sync.dma_start(bv_bc, _bcast(bv[:]))
            nc.sync.dma_start(bo_bc, _bcast(bo[:]))
            valid_sb = const.tile([128, NT_KV], F32, tag="vld")
            nc.sync.dma_start(valid_sb, valid[:].rearrange("(b p) -> p b", p=128))

            # identity for PE transpose (bf16)
            ident = const.tile([128, 128], BF16, tag="ident")
            nc.gpsimd.memset(ident, 0.0)
            nc.gpsimd.affine_select(
                out=ident, in_=ident,
                compare_op=mybir.AluOpType.not_equal, fill=1.0,
                base=0, pattern=[[-1, 128]], channel_multiplier=1,
            )

            # constant band masks for window blocks 0 and 2 (block 1 is
            # fully in-band): [k-in-block(128), (w in {0,2}, q)(2x128)] bf16
            bmask = const.tile([128, 2, 128], BF16, tag="bmask")
            nc.gpsimd.memset(bmask, 1.0)
            nc.gpsimd.affine_select(
                out=bmask[:, 0], in_=bmask[:, 0],
                compare_op=mybir.AluOpType.is_ge, fill=0.0,
                base=0, pattern=[[-1, 128]], channel_multiplier=1,
            )
            nc.gpsimd.affine_select(
                out=bmask[:, 1], in_=bmask[:, 1],
                compare_op=mybir.AluOpType.is_ge, fill=0.0,
                base=0, pattern=[[1, 128]], channel_multiplier=-1,
            )

            # ---- projections ----
            QT = acts.tile([128, NFT, HALF], BF16, tag="QT")
            KT = acts.tile([128, NFT, KV], BF16, tag="KT")
            VST = acts.tile([128, NT_KV, H * HV], BF16, tag="VST")

            for x_sb, w_sb, o_sb, b_sb, n_s in (
                (xq_sb, wq_sb, QT, bq_sb, HALF),
                (xk_sb, wk_sb, KT, bk_sb, KV),
            ):
                for ft in range(NFT):
                    for c0 in range(0, n_s, 512):
                        cw = min(512, n_s - c0)
                        acc = ps.tile([128, 512], F32, tag="ps_big", bufs=2)
                        for dt_i in range(NFT):
                            nc.tensor.matmul(
                                acc[:, :cw],
                                w_sb[:, dt_i, ft * 128:(ft + 1) * 128],
                                x_sb[:, dt_i, c0:c0 + cw],
                                start=(dt_i == 0),
                                stop=(dt_i == NFT - 1),
                            )
                        nc.vector.tensor_scalar_add(
                            o_sb[:, ft, c0:c0 + cw], acc[:, :cw], b_sb[:, ft:ft + 1]
                        )

            for blk in range(NT_KV):
                for hc in range(2):
                    c0 = hc * (H * HV // 2)  # 260-wide halves
                    cw = H * HV // 2
                    acc = ps.tile([128, 260], F32, tag="ps_big", bufs=2, name="acc")
                    for dt_i in range(NFT):
                        nc.tensor.matmul(
                            acc,
                            xv_sb[:, dt_i, blk * 128:(blk + 1) * 128],
                            wv_sb[:, dt_i, c0:c0 + cw],
                            start=(dt_i == 0),
                            stop=(dt_i == NFT - 1),
                        )
                    nc.vector.tensor_add(
                        VST[:, blk, c0:c0 + cw], acc, bv_bc[:, c0:c0 + cw]
                    )
                # validity column (per head): col h*HV + 64
                vcol = VST[:, blk].rearrange("p (h c) -> p h c", c=HV)[:, :, DH:DH + 1]
                nc.vector.tensor_scalar_add(vcol, vcol, valid_sb[:, blk:blk + 1])

            # ---- attention: q-tiles of 128, window = kv blocks t, t+1, t+2 ----
            for t in range(NTQ):
                attn_sb = work.tile([128, D], BF16, tag="attn", bufs=4, name="attn")
                for hp in range(H // 2):  # head pairs (2hp, 2hp+1) share ft plane
                    ft = hp
                    sts = []
                    for hs in range(2):
                        sts.append(ps.tile([128, 384], F32, tag="ps_st", bufs=3,
                                           name="st"))
                    # paired score MMs: rows 0-63 and 64-127
                    for w in range(3):
                        for hs in range(2):
                            p0 = hs * 64
                            nc.tensor.matmul(
                                sts[hs][:, w * 128:(w + 1) * 128],
                                KT[p0:p0 + 64, ft, (t + w) * 128:(t + w + 1) * 128],
                                QT[p0:p0 + 64, ft, t * 128:(t + 1) * 128],
                                start=True,
                                stop=True,
                                tile_position=(p0, 0),
                            )
                    aps2 = ps.tile([128, 2 * HV], F32, tag="ps_sm", bufs=3,
                                   name="aps2")
                    for hs in range(2):
                        h = 2 * hp + hs
                        pt = work.tile([128, 384], BF16, tag="pt", bufs=8)
                        nc.scalar.activation(
                            pt, sts[hs], mybir.ActivationFunctionType.Exp,
                            scale=SCALE,
                        )
                        ptv = pt.rearrange("p (w j) -> p w j", j=128)[:, 0:3:2, :]
                        if hs == 0:
                            nc.vector.tensor_mul(ptv, ptv, bmask)
                        else:
                            nc.gpsimd.tensor_mul(ptv, ptv, bmask)
                        for w in range(3):
                            nc.tensor.matmul(
                                aps2[:, hs * HV:hs * HV + HV],
                                pt[:, w * 128:(w + 1) * 128],
                                VST[:, t + w, h * HV:(h + 1) * HV],
                                start=(w == 0),
                                stop=(w == 2),
                            )
                    # normalize both heads: one batched reciprocal, two muls
                    rec2 = work.tile([128, 2], F32, tag="rec", bufs=8)
                    dn = aps2.rearrange("p (h c) -> p h c", c=HV)[:, :, DH:DH + 1]
                    nc.vector.reciprocal(rec2, dn)
                    for hs in range(2):
                        h = 2 * hp + hs
                        nc.vector.tensor_scalar_mul(
                            attn_sb[:, h * DH:(h + 1) * DH],
                            aps2[:, hs * HV:hs * HV + DH], rec2[:, hs:hs + 1]
                        )
                # transpose attn + output projection
                atT = []
                for ft in range(NFT):
                    tp = ps.tile([128, 128], BF16, tag="ps_sm", bufs=2, name="tp")
                    nc.tensor.transpose(tp, attn_sb[:, ft * 128:(ft + 1) * 128], ident)
                    at = work.tile([128, 128], BF16, tag="atT", bufs=8, name="at")
                    nc.vector.tensor_copy(at, tp)
                    atT.append(at)
                ops = ps.tile([128, D], F32, tag="ps_big", bufs=2, name="ops")
                for ft in range(NFT):
                    nc.tensor.matmul(
                        ops, atT[ft], wo_sb[:, ft, :],
                        start=(ft == 0), stop=(ft == NFT - 1),
                    )
                ob = work.tile([128, D], F32, tag="ob", bufs=3)
                nc.vector.tensor_add(ob, ops, bo_bc)
                nc.sync.dma_start(out[t * 128:(t + 1) * 128, :], ob)

    nc.compile()
    return nc


_CACHE = threading.Lock(), {}


def _get_nc():
    lock, cache = _CACHE
    with lock:
        if "nc" not in cache:
            cache["nc"] = build_nc()
        return cache["nc"]


def _prep_core_inputs(b, h2, query, key, value, shared):
    qs = h2 * HALF
    xqT = np.ascontiguousarray(query[b, qs:qs + HALF, :].T.astype(NP_BF16))

    lo, hi = qs - ATTN, qs + HALF + ATTN
    clo, chi = max(lo, 0), min(hi, K)
    pad_front, pad_back = clo - lo, hi - chi
    kpad = np.zeros((KV, D), NP_BF16)
    vpad = np.zeros((KV, D), NP_BF16)
    kpad[pad_front:KV - pad_back] = key[b, clo:chi, :].astype(NP_BF16)
    vpad[pad_front:KV - pad_back] = value[b, clo:chi, :].astype(NP_BF16)
    val = np.zeros(KV, np.float32)
    val[pad_front:KV - pad_back] = 1.0

    m = {
        "xqT": xqT,
        "xkT": np.ascontiguousarray(kpad.T),
        "xvT": np.ascontiguousarray(vpad.T),
        "valid": val,
    }
    m.update(shared)
    return m


def _prep_shared(Wq, bq, Wk, bk, Wv, bv, Wo, bo):
    wvT = np.asarray(Wv, np.float32).T  # [D, H*DH]
    wvT_aug = np.zeros((D, H * HV), np.float32)
    bv_aug = np.zeros(H * HV, np.float32)
    for h in range(H):
        wvT_aug[:, h * HV:h * HV + DH] = wvT[:, h * DH:(h + 1) * DH]
        bv_aug[h * HV:h * HV + DH] = np.asarray(bv, np.float32)[h * DH:(h + 1) * DH]
    return {
        "wqT": np.ascontiguousarray(np.asarray(Wq, np.float32).T.astype(NP_BF16)),
        "wkT": np.ascontiguousarray(np.asarray(Wk, np.float32).T.astype(NP_BF16)),
        "wvT": wvT_aug.astype(NP_BF16),
        "woT": np.ascontiguousarray(np.asarray(Wo, np.float32).T.astype(NP_BF16)),
        "bq": np.asarray(bq, np.float32),
        "bk": np.asarray(bk, np.float32),
        "bv": bv_aug,
        "bo": np.asarray(bo, np.float32),
    }


def kernel(query, key, value, Wq, bq, Wk, bk, Wv, bv, Wo, bo):
    query = np.asarray(query, np.float32)
    key = np.asarray(key, np.float32)
    value = np.asarray(value, np.float32)
    shared = _prep_shared(Wq, bq, Wk, bk, Wv, bv, Wo, bo)

    in_maps = []
    for core in range(NCORES):
        b, h2 = divmod(core, 2)
        in_maps.append(_prep_core_inputs(b, h2, query, key, value, shared))

    nc = _get_nc()
    res = run_bass_kernel_spmd(nc, in_maps, core_ids=list(range(NCORES)))

    full = np.empty((B, K, D), np.float32)
    for core in range(NCORES):
        b, h2 = divmod(core, 2)
        full[b, h2 * HALF:(h2 + 1) * HALF, :] = res.results[core]["out"]
    return full
